# revision 5
# baseline (speedup 1.0000x reference)
"""Trainium2 Bass kernel for nn_DimBlock_1 (light-field 4D conv -> 2D conv).

Math: out[b, oc, h, w] = bias[oc] +
      sum_{ic<25, kh<9, kw<9} pic[b, ic, h+kh, w+kw] * W[oc, ic, kh, kw]
with pic [8, 25, 256, 256] (25 = 5x5 angular dims folded to channels),
W [100, 25, 9, 9], output [8, 100, 1, 1, 248, 248].

Strategy (pure data parallel, 1 image per NeuronCore):
- Flatten the image spatially: free dim = h*256+w. Every (kh, kw) kernel tap
  is then just a free-dim offset of kh*256+kw into the same SBUF tile.
- Pack the contraction: partitions hold 5 shifted copies of the 25 channels
  (group g = image shifted by +g elements), so one K=125 matmul covers 5
  consecutive kw taps. 81 taps = 9 kh x (5+4 kw) = 18 accumulating matmuls
  per PSUM tile (second half per kh has zero weights in group 4).
- float32r matmul: full PE rate at N=512 with ~1.4e-4 max rel error.
- Compute full 256-wide rows (248 valid + 8 overcompute), evict PSUM via
  ScalarE Identity+bias into SBUF staging, DMA out only the valid 248 cols.
"""

import sys

sys.path.insert(0, "/opt/trn_rl_repo")

import numpy as np

from concourse import bacc
import concourse.tile as tile
import concourse.mybir as mybir
from concourse.bass_utils import run_bass_kernel_spmd

B, C, H, W = 8, 25, 256, 256
OC, KH, KW = 100, 9, 9
OH, OW = H - KH + 1, W - KW + 1  # 248, 248
NCORES = 8
NPIX = H * W

STRIP = 16              # output rows per strip
NMM = 18                # matmuls per psum tile: 9 kh x 2 kw-halves
KP = 125                # contraction partitions: 25 ch x 5 shift groups
LP = STRIP * W + 8 * W + 8  # sbuf free size per input tile

F32 = mybir.dt.float32
F32R = mybir.dt.float32r

_compiled = None


def _build():
    nc = bacc.Bacc("TRN2", target_bir_lowering=False, debug=False,
                   num_devices=NCORES)
    pic = nc.dram_tensor("pic", [C, NPIX], F32R, kind="ExternalInput").ap()
    wp = nc.dram_tensor("wp", [KP, NMM, OC], F32R, kind="ExternalInput").ap()
    bias = nc.dram_tensor("bias", [OC, 1], F32, kind="ExternalInput").ap()
    out = nc.dram_tensor("out", [OC, OH, OW], F32, kind="ExternalOutput").ap()

    with tile.TileContext(nc) as tc:
        with (
            tc.tile_pool(name="wpool", bufs=1) as wpool,
            tc.tile_pool(name="inpool", bufs=3) as inpool,
            tc.tile_pool(name="outpool", bufs=3) as outpool,
            tc.tile_pool(name="pspool", bufs=8, space="PSUM") as pspool,
        ):
            # weights/bias on the gpsimd queue: loads in parallel with the
            # first strip's input DMAs on sync/vector
            wt = wpool.tile([KP, NMM, OC], F32R)
            nc.gpsimd.dma_start(wt[:], wp[:])
            bt = wpool.tile([OC, 1], F32)
            nc.gpsimd.dma_start(bt[:], bias[:])

            # small first/last strips shorten pipeline fill and drain
            strip_sizes = [4] + [STRIP] * ((OH - 8) // STRIP) + [4]
            assert sum(strip_sizes) == OH
            h0 = 0
            for si, rows in enumerate(strip_sizes):
                base = h0 * W
                need = rows * W + 8 * W + 8
                xt = inpool.tile([128, LP], F32R, tag="xt")
                for g in range(5):
                    ln = min(need, NPIX - base - g)
                    eng = (nc.sync, nc.scalar)[g % 2] if si == 0 else nc.sync
                    eng.dma_start(xt[25 * g:25 * g + 25, 0:ln],
                                  pic[:, base + g:base + g + ln])
                ot = outpool.tile([OC, STRIP * W], F32, tag="ot")
                for t in range(rows * W // 512):
                    pt = pspool.tile([OC, 512], F32, tag="pt")
                    for j in range(NMM):
                        kh, kwb = j // 2, (j % 2) * 5
                        o = t * 512 + kh * W + kwb
                        nc.tensor.matmul(pt[:], wt[:, j, :],
                                         xt[0:KP, o:o + 512],
                                         start=(j == 0), stop=(j == NMM - 1))
                    nc.scalar.activation(
                        ot[:, t * 512:(t + 1) * 512], pt[:],
                        mybir.ActivationFunctionType.Identity, bias=bt[:])
                src = ot[:, :rows * W].rearrange("p (r w) -> p r w", w=W)
                # gpsimd queue: keeps output drains off the Sync queue so the
                # next strip's input DMAs are not serialized behind them
                nc.gpsimd.dma_start(out[:, h0:h0 + rows, :], src[:, :, :OW])
                h0 += rows

    nc.compile()
    return nc


def _pack_weights(weight: np.ndarray) -> np.ndarray:
    w2 = np.ascontiguousarray(weight.reshape(OC, C, KH, KW))
    wp = np.zeros((KP, NMM, OC), dtype=np.float32)
    for kh in range(KH):
        for half in range(2):
            j, kwb = kh * 2 + half, half * 5
            for g in range(5):
                kw = kwb + g
                if kw < KW:
                    wp[25 * g:25 * g + 25, j, :] = w2[:, :, kh, kw].T
    return wp


def _run(pic_in, weight, bias, trace=False):
    global _compiled
    if _compiled is None:
        _compiled = _build()
    nc = _compiled
    wp = _pack_weights(np.asarray(weight, dtype=np.float32))
    bvec = np.ascontiguousarray(
        np.asarray(bias, dtype=np.float32).reshape(OC, 1))
    pic = np.asarray(pic_in, dtype=np.float32).reshape(B, C, NPIX)
    in_maps = [
        {"pic": np.ascontiguousarray(pic[i]), "wp": wp, "bias": bvec}
        for i in range(NCORES)
    ]
    res = run_bass_kernel_spmd(nc, in_maps, core_ids=list(range(NCORES)),
                               trace=trace)
    full = np.stack([res.results[i]["out"] for i in range(NCORES)], axis=0)
    return full.reshape(B, OC, 1, 1, OH, OW), res


def kernel(pic_in, weight, bias):
    out, _ = _run(pic_in, weight, bias, trace=False)
    return out


def kernel_traced(pic_in, weight, bias):
    return _run(pic_in, weight, bias, trace=True)
